# revision 30
# baseline (speedup 1.0000x reference)
"""Trainium2 Bass kernel for nn_BaseLayerGate (MoE balanced routing).

8 NeuronCores, data-parallel over tokens (2048 tokens/core):

  - Affinity matmul aff^T = centT.T @ featsT on the tensor engine in fp32
    (exact: the Sinkhorn top-cap ordering is extremely tie-sensitive, so the
    fast reduced-precision PE modes flip too many assignments on hardware).
    A short warm-up dummy-matmul burst raises the PE clock out of its low
    p-state before the first real tile arrives.
  - Sinkhorn (10 iters) in reciprocal-potential form:
      R[n]  = sum_se E0[n,se] * V[se]     (PE matvec, V = slot-masked 1/C)
      C[se] = sum_n  E0[n,se] * W[n]      (PE matvec accum, W = 1/R)
    The token-direction C-sum is global: per-expert partials are all-reduced
    across the 8 cores with a 3-stage XOR-butterfly implemented with
    remote_dma_broadcast (SBUF->SBUF, XOR-relative dests) + trigger_dma —
    ~1us per exchange vs ~21us for a collective_compute AllGather.
    Remote sems are parity-split (round % 2) and send/recv buffers parity
    double-buffered so the race detector's sem-watermark rules are satisfied
    without extra ack messages.
  - exp runs in 4 column chunks so the first R-step can start right after the
    last affinity tile instead of after a full-width activation pass.
  - Outputs: aff^T and the final raw R (10th R-step). The per-column-constant
    subtraction Z = aff - ln(R) and the per-expert top-cap selection happen on
    the host (ln taken in fp64 there, more accurate than an on-chip Ln LUT).
"""

import numpy as np

import concourse.bass as bass
from concourse import mybir
from concourse.bass_utils import run_bass_kernel_spmd

N_CORES = 8
N = 16384
D = 1024
KSLOT = 2
E = 64
SE = KSLOT * E
CAP = N // E
TOK = N // N_CORES
ITERS = 10
STAGES = [1, 2, 4]  # XOR-butterfly partner offsets
KORDER_A = [2, 0, 1, 5, 3]  # phase A: k-outer, ~DMA arrival order
KORDER_B = [4, 6, 7]  # phase B: n-outer so n-blocks finish early
N_WARM = 14  # PE p-state warm-up dummies

F32 = mybir.dt.float32

# ---- engine semaphore schedules (every op increments its engine's sem by 1
# so the OOO engine queues are race-free under the sem-watermark detector) ---

# PE program order: [phase A: no incs], n0B=1, n1B=2, [t0,t1,R0g0,t2,t3]=3..6,
# n2B=7, [t4,t5,R0g1,t6,t7]=8..11, n3B=12, [t8,t9,R0g2,t10,t11]=13..16,
# R0g3=17, t12..t15=18..21, C0=22, then R/C pairs
P_AFFN = {0: 1, 1: 2, 2: 7, 3: 16}
P_TD = {0: 3, 1: 4, 2: 5, 3: 6, 4: 8, 5: 9, 6: 10, 7: 11, 8: 12, 9: 13,
        10: 14, 11: 15, 12: 18, 13: 19, 14: 20, 15: 21}
P_R0 = 17
def P_R(r):
    return P_R0 if r == 0 else 21 + 2 * r
def P_C(r):
    return 22 + 2 * r

# DVE per round r: W=7r+1, cp=7r+2, adds=7r+3..5, recipVa/b=7r+6,7r+7
def V_W(r):
    return 7 * r + 1
def V_CP(r):
    return 7 * r + 2
def V_S(r, j):  # dve value at which stage-j send buffer of round r is ready
    return V_CP(r) + j
def V_ADD(r, j):
    return 7 * r + 3 + j
def V_V(r):
    return 7 * r + 7
V_RSUM = V_V(8) + 1  # 64


def _build_nc():
    nc = bass.Bass()

    featsT_in = nc.declare_dram_parameter("featsT", [D, TOK], F32, isOutput=False)
    centT_in = nc.declare_dram_parameter("centT", [D, SE], F32, isOutput=False)
    v0_in = nc.declare_dram_parameter("v0", [SE, 2], F32, isOutput=False)
    ident_in = nc.declare_dram_parameter("ident", [128, 128], F32, isOutput=False)

    afft_out = nc.declare_dram_parameter("afft", [SE, TOK], F32, isOutput=True)
    rsum_out = nc.declare_dram_parameter("rsum", [128, 32], F32, isOutput=True)

    from contextlib import ExitStack

    es = ExitStack()
    featsT_sb = es.enter_context(nc.sbuf_tensor("featsT_sb", [128, 8, TOK], F32))
    centT_sb = es.enter_context(nc.sbuf_tensor("centT_sb", [128, 8, SE], F32))
    e0t_sb = es.enter_context(nc.sbuf_tensor("e0t_sb", [128, TOK], F32))
    e0tm_sb = es.enter_context(nc.sbuf_tensor("e0tm_sb", [128, 16, 128], F32))
    ident_sb = es.enter_context(nc.sbuf_tensor("ident_sb", [128, 128], F32))
    v_sb = es.enter_context(nc.sbuf_tensor("v_sb", [128, 2], F32))
    w_sb = es.enter_context(nc.sbuf_tensor("w_sb", [128, 16, 2], F32))
    affT_sb = es.enter_context(nc.sbuf_tensor("affT_sb", [128, TOK], F32))
    rsum_sb = es.enter_context(nc.sbuf_tensor("rsum_sb", [128, 32], F32))
    s_sb = es.enter_context(nc.sbuf_tensor("s_sb", [128, 12], F32))  # [stage*4 + r%4]
    r_sb = es.enter_context(nc.sbuf_tensor("r_sb", [128, 6], F32))
    tot_sb = es.enter_context(nc.sbuf_tensor("tot_sb", [128, 1], F32))

    ps_aff = es.enter_context(nc.psum_tensor("ps_aff", [128, TOK], F32))
    ps_r = es.enter_context(nc.psum_tensor("ps_r", [128, 32], F32))
    ps_c = es.enter_context(nc.psum_tensor("ps_c", [128, 2], F32))
    ps_tp = es.enter_context(nc.psum_tensor("ps_tp", [128, 128], F32))
    ps_tp2 = es.enter_context(nc.psum_tensor("ps_tp2", [128, 128], F32))

    block = es.enter_context(nc.Block())
    in_sem = es.enter_context(nc.semaphore("in_sem"))
    out_sem = es.enter_context(nc.semaphore("out_sem"))
    fsems = [es.enter_context(nc.semaphore(f"fsem{k}")) for k in range(8)]
    pe_sem = es.enter_context(nc.semaphore("pe_sem"))
    act_sem = es.enter_context(nc.semaphore("act_sem"))
    dve_sem = es.enter_context(nc.semaphore("dve_sem"))
    psem = es.enter_context(nc.semaphore("psem"))
    exp_sems = [es.enter_context(nc.semaphore(f"exp{n}")) for n in range(4)]
    csems = [es.enter_context(nc.semaphore(f"cs{t}")) for t in range(16)]
    afcs = [es.enter_context(nc.semaphore(f"afc{n}")) for n in range(4)]
    lsem = es.enter_context(nc.semaphore("lsem"))
    ack_sem = es.enter_context(nc.semaphore("ack_sem"))
    rsems = [
        [es.enter_context(nc.semaphore(f"rsem{j}_{p}")) for p in range(2)]
        for j in range(3)
    ]

    with es:
        # -------- sync engine (SP): v0/ident + feats k0,k3 + output DMAs ----
        @block.sync
        def _(eng):
            eng.dma_start(out=v_sb[:], in_=v0_in[:]).then_inc(in_sem, 16)
            eng.dma_start(out=ident_sb[:], in_=ident_in[:]).then_inc(in_sem, 16)
            for k in (0, 3):
                eng.dma_start(
                    out=featsT_sb[:, k, :], in_=featsT_in[128 * k : 128 * (k + 1), :]
                ).then_inc(fsems[k], 16)
            eng.wait_ge(afcs[0], 1)
            eng.dma_start(out=afft_out[:], in_=affT_sb[:]).then_inc(out_sem, 16)
            eng.wait_ge(dve_sem, V_RSUM)
            eng.dma_start(out=rsum_out[:], in_=rsum_sb[:]).then_inc(out_sem, 16)
            eng.wait_ge(out_sem, 32)

        # -------- scalar (ACT): centT + feats k1,k4 + exp + afft chunks -----
        @block.scalar
        def _(eng):
            src_ap = centT_in.ap().rearrange("(k p) e -> p k e", p=128)
            with nc.allow_non_contiguous_dma(reason="512B-run k-major centT load"):
                eng.dma_start(out=centT_sb[:], in_=src_ap).then_inc(in_sem, 16)
            for k in (1, 4):
                eng.dma_start(
                    out=featsT_sb[:, k, :], in_=featsT_in[128 * k : 128 * (k + 1), :]
                ).then_inc(fsems[k], 16)
            for n in range(4):  # exp chunks chasing phase-B blocks
                eng.wait_ge(pe_sem, P_AFFN[n])
                eng.activation(
                    e0t_sb[:, 512 * n : 512 * (n + 1)],
                    ps_aff[:, 512 * n : 512 * (n + 1)],
                    mybir.ActivationFunctionType.Exp,
                ).then_inc(exp_sems[n], 1)
            for t in (13, 15):  # late e0tm copies (GPSIMD cannot touch PSUM)
                eng.wait_ge(pe_sem, P_TD[t])
                eng.activation(
                    e0tm_sb[:, t, :], ps_tp2[:], mybir.ActivationFunctionType.Copy
                ).then_inc(csems[t], 1)
            eng.activation(
                affT_sb[:], ps_aff[:, 0:TOK], mybir.ActivationFunctionType.Copy
            ).then_inc(afcs[0], 1)

        # -------- tensor engine (PE) ----------------------------------------
        @block.tensor
        def _(eng):
            # p-state warm-up on tiny tiles while feats DMAs are in flight
            eng.wait_ge(in_sem, 48)  # v0 + ident + centT
            for _ in range(N_WARM):
                eng.matmul(ps_r[0:2, 0:32], v_sb[:], ident_sb[:, 0:32],
                           start=True, stop=True)

            # phase A: k-outer over the first 5 chunks
            for i, k in enumerate(KORDER_A):
                eng.wait_ge(fsems[k], 16)
                for n in range(4):
                    eng.matmul(
                        ps_aff[:, 512 * n : 512 * (n + 1)],
                        centT_sb[:, k, :],
                        featsT_sb[:, k, 512 * n : 512 * (n + 1)],
                        start=(i == 0),
                        stop=False,
                    )
            for k in KORDER_B:
                eng.wait_ge(fsems[k], 16)

            def nblock(n):
                for j, k in enumerate(KORDER_B):
                    mm = eng.matmul(
                        ps_aff[:, 512 * n : 512 * (n + 1)],
                        centT_sb[:, k, :],
                        featsT_sb[:, k, 512 * n : 512 * (n + 1)],
                        start=False,
                        stop=(j == len(KORDER_B) - 1),
                    )
                mm.then_inc(pe_sem, 1)  # P_AFFN[n]

            def transpose(t, buf, wait=None):
                if wait is not None:
                    eng.wait_ge(*wait)
                eng.transpose(
                    buf, e0t_sb[:, 128 * t : 128 * (t + 1)], ident_sb[:]
                ).then_inc(pe_sem, 1)  # P_TD[t]

            def r_tile(t):
                return eng.matmul(
                    ps_r[:, 2 * t : 2 * (t + 1)],
                    e0t_sb[:, 128 * t : 128 * (t + 1)],
                    v_sb[:],
                    start=True,
                    stop=True,
                )

            def r_group(g):
                for t in range(4 * g, 4 * g + 4):
                    mm = r_tile(t)
                return mm

            # transpose buffers: tp/tp2 + freed ps_aff bank slices (the afft
            # chunk copies retire each 512-col bank for reuse as scratch)
            def tbuf(t):
                return ps_tp[:] if t % 2 == 0 else ps_tp2[:]

            # phase B: n-outer; transposes + R0 groups slotted into the gaps
            nblock(0)
            nblock(1)
            eng.wait_ge(exp_sems[0], 1)
            transpose(0, tbuf(0))
            transpose(1, tbuf(1))
            r_group(0)
            transpose(2, tbuf(2), (csems[0], 1))
            transpose(3, tbuf(3), (csems[1], 1))
            nblock(2)
            eng.wait_ge(exp_sems[1], 1)
            transpose(4, tbuf(4), (csems[2], 1))
            transpose(5, tbuf(5), (csems[3], 1))
            r_group(1)
            transpose(6, tbuf(6), (csems[4], 1))
            transpose(7, tbuf(7), (csems[5], 1))
            eng.wait_ge(exp_sems[2], 1)
            transpose(8, tbuf(8), (csems[6], 1))
            transpose(9, tbuf(9), (csems[7], 1))
            r_group(2)
            transpose(10, tbuf(10), (csems[8], 1))
            transpose(11, tbuf(11), (csems[9], 1))
            nblock(3)
            eng.wait_ge(exp_sems[3], 1)
            r_group(3).then_inc(pe_sem, 1)  # P_R0
            for t in range(12, 16):
                transpose(t, tbuf(t), (csems[t - 2], 1))

            def c_step(r):
                # slot-packed C: region [0:64] col 0 <- slot 0, [64:128] <- slot 1
                for lo, hi, s in ((0, 64, 0), (64, 128, 1)):
                    for t in range(16):
                        if r == 0 and t >= 12:
                            eng.wait_ge(csems[t], 1)
                        mm = eng.matmul(
                            ps_c[lo:hi, 0:1],
                            e0tm_sb[:, t, lo:hi],
                            w_sb[:, t, s : s + 1],
                            start=(t == 0),
                            stop=(t == 15),
                        )
                return mm

            def r_step():
                for t in range(16):
                    mm = r_tile(t)
                return mm

            for r in range(ITERS - 1):
                eng.wait_ge(dve_sem, V_W(r))
                c_step(r).then_inc(pe_sem, 1)  # P_C(r)
                eng.wait_ge(dve_sem, V_V(r))
                r_step().then_inc(pe_sem, 1)  # P_R(r+1)

        # -------- vector (DVE): e0tm copies, recips, butterfly adds ---------
        @block.vector
        def _(eng):
            def cbuf(t):
                return ps_tp[:] if t % 2 == 0 else ps_tp2[:]

            for t in range(12):  # early-tile e0tm copies
                eng.wait_ge(pe_sem, P_TD[t])
                eng.tensor_copy(e0tm_sb[:, t, :], cbuf(t)).then_inc(csems[t], 1)
            # W0 = 1/R0
            eng.wait_ge(pe_sem, P_R0)
            eng.reciprocal(
                w_sb.ap().rearrange("p t s -> p (t s)"), ps_r[:, 0:32]
            ).then_inc(dve_sem, 1)  # V_W(0)
            for t in (12, 14):  # late copies
                eng.wait_ge(pe_sem, P_TD[t])
                eng.tensor_copy(e0tm_sb[:, t, :], ps_tp[:]).then_inc(csems[t], 1)

            for r in range(ITERS - 1):
                par = r % 2
                par4 = r % 4
                if r >= 1:
                    eng.wait_ge(pe_sem, P_R(r))
                    eng.reciprocal(
                        w_sb.ap().rearrange("p t s -> p (t s)"), ps_r[:, 0:32]
                    ).then_inc(dve_sem, 1)  # V_W(r)
                # cpart -> s0 (slot-packed by the C-step already)
                eng.wait_ge(pe_sem, P_C(r))
                if r >= 4:
                    eng.wait_ge(ack_sem, (r - 2) // 2)  # rounds <= r-4 drained
                eng.tensor_copy(s_sb[:, par4 : par4 + 1], ps_c[:, 0:1]).then_inc(
                    dve_sem, 1
                )  # V_CP(r)
                for j in range(3):
                    eng.wait_ge(dve_sem, V_S(r, j))
                    eng.wait_ge(rsems[j][par], 2 * (r // 2 + 1))
                    dst = (
                        s_sb[:, 4 * (j + 1) + par4 : 4 * (j + 1) + par4 + 1]
                        if j < 2
                        else tot_sb[:]
                    )
                    eng.tensor_add(
                        dst,
                        s_sb[:, 4 * j + par4 : 4 * j + par4 + 1],
                        r_sb[:, 2 * j + par : 2 * j + par + 1],
                    ).then_inc(dve_sem, 1)  # V_ADD(r, j)
                # V = slot-masked 1/total
                eng.wait_ge(dve_sem, V_ADD(r, 2))
                eng.reciprocal(v_sb[0:64, 0:1], tot_sb[0:64, :]).then_inc(dve_sem, 1)
                eng.wait_ge(dve_sem, V_V(r) - 1)
                eng.reciprocal(v_sb[64:128, 1:2], tot_sb[64:128, :]).then_inc(
                    dve_sem, 1
                )  # V_V(r)

            eng.wait_ge(pe_sem, P_R(ITERS - 1))
            eng.tensor_copy(rsum_sb[:], ps_r[:, 0:32]).then_inc(dve_sem, 1)  # V_RSUM

        # -------- gpsimd (Pool): feats k2,k5,k6,k7 + late copies + butterfly
        @block.gpsimd
        def _(eng):
            from concourse import library_config

            for k in (2, 5, 6, 7):
                eng.dma_start(
                    out=featsT_sb[:, k, :], in_=featsT_in[128 * k : 128 * (k + 1), :]
                ).then_inc(fsems[k], 16)
            eng.load_library(library_config.remote_dma)
            nprep = 0
            for r in range(ITERS - 1):
                par = r % 2
                for j, d in enumerate(STAGES):
                    rdests = [None] * 8
                    rdests[d] = (0, d)
                    if nprep >= 14:  # SWDGE ring backpressure (66 descs/entry)
                        eng.wait_ge(ack_sem, -(-(nprep - 13) // 6))
                    eng.remote_dma_broadcast(
                        out_ap=r_sb[:, 2 * j + par : 2 * j + par + 1],
                        in_ap=s_sb[:, 4 * j + r % 4 : 4 * j + r % 4 + 1],
                        remote_sem=rsems[j][par],
                        local_sem=lsem,
                        rdests=rdests,
                    ).then_inc(psem, 1)
                    nprep += 1
                    eng.wait_ge(psem, nprep)
                    eng.wait_ge(dve_sem, V_S(r, j))
                    eng.trigger_dma(1)
                    if j == 2 and r % 2 == 1:
                        # both rounds of this parity pair drained -> ack
                        eng.wait_ge(lsem, 16 * nprep).then_inc(ack_sem, 1)

    return nc


_CACHE = {}


def _get_nc():
    if "nc" not in _CACHE:
        nc = _build_nc()
        # Raw Bass skips Bacc's codegen_inst_isa pass; without it the NEFF
        # compiler sees empty .instr on the extended-ISA (remote DMA, library
        # load) instructions and fails with "ISA wrong length".
        from concourse.library_overlay import lower_extended_insts

        lower_extended_insts(nc)
        _CACHE["nc"] = nc
    return _CACHE["nc"]


def make_in_maps(input_features, expert_centroids):
    feats = np.ascontiguousarray(
        np.asarray(input_features, dtype=np.float32).reshape(-1, D)
    )
    cent = np.asarray(expert_centroids, dtype=np.float32).reshape(SE, D)

    featsT = np.ascontiguousarray(feats.T)
    centT = np.ascontiguousarray(cent.T)
    ident = np.eye(128, dtype=np.float32)
    v0 = np.zeros((SE, 2), np.float32)
    v0[0:64, 0] = 1.0
    v0[64:128, 1] = 1.0

    in_maps = []
    for c in range(N_CORES):
        in_maps.append(
            {
                "featsT": np.ascontiguousarray(featsT[:, TOK * c : TOK * (c + 1)]),
                "centT": centT,
                "ident": ident,
                "v0": v0,
            }
        )
    return in_maps


def kernel(input_features: np.ndarray, expert_centroids: np.ndarray):
    in_maps = make_in_maps(input_features, expert_centroids)
    nc = _get_nc()
    res = run_bass_kernel_spmd(nc, in_maps, list(range(N_CORES)))

    afft = np.concatenate([res.results[c]["afft"] for c in range(N_CORES)], axis=1)
    # rsum[c][p, 2t+s] = R_s[token c*2048 + t*128 + p]
    lnr = np.empty((KSLOT, N), np.float64)
    for c in range(N_CORES):
        rs = np.asarray(res.results[c]["rsum"], np.float64).reshape(128, 16, 2)
        for s in range(KSLOT):
            lnr[s, c * TOK : (c + 1) * TOK] = np.log(rs[:, :, s]).T.reshape(-1)

    A = afft.reshape(KSLOT, E, N)
    Z = A.astype(np.float64) - lnr[:, None, :]
    idx = np.empty((KSLOT, E, CAP), np.int32)
    vals = np.empty((KSLOT, E, CAP), np.float32)
    for k in range(KSLOT):
        for e in range(E):
            col = Z[k, e]
            part = np.sort(np.argpartition(-col, CAP - 1)[:CAP])
            order = part[np.argsort(-col[part], kind="stable")]
            idx[k, e] = order.astype(np.int32)
            vals[k, e] = A[k, e, order]
    return idx, vals


# revision 34
# speedup vs baseline: 1.0240x; 1.0240x over previous
"""Trainium2 Bass kernel for nn_BaseLayerGate (MoE balanced routing).

8 NeuronCores, data-parallel over tokens (2048 tokens/core):

  - Affinity matmul aff^T = centT.T @ featsT on the tensor engine in fp32
    (exact: the Sinkhorn top-cap ordering is extremely tie-sensitive, so the
    fast reduced-precision PE modes flip too many assignments on hardware).
    A short warm-up dummy-matmul burst raises the PE clock out of its low
    p-state before the first real tile arrives.
  - Sinkhorn (10 iters) in reciprocal-potential form:
      R[n]  = sum_se E0[n,se] * V[se]     (PE matvec, V = slot-masked 1/C)
      C[se] = sum_n  E0[n,se] * W[n]      (PE matvec accum, W = 1/R)
    The token-direction C-sum is global: per-expert partials are all-reduced
    across the 8 cores with a 3-stage XOR-butterfly implemented with
    remote_dma_broadcast (SBUF->SBUF, XOR-relative dests) + trigger_dma —
    ~1us per exchange vs ~21us for a collective_compute AllGather.
    Remote sems are parity-split (round % 2) and send/recv buffers parity
    double-buffered so the race detector's sem-watermark rules are satisfied
    without extra ack messages.
  - exp runs in 4 column chunks so the first R-step can start right after the
    last affinity tile instead of after a full-width activation pass.
  - Outputs: aff^T and the final raw R (10th R-step). The per-column-constant
    subtraction Z = aff - ln(R) and the per-expert top-cap selection happen on
    the host (ln taken in fp64 there, more accurate than an on-chip Ln LUT).
"""

import numpy as np

import concourse.bass as bass
from concourse import mybir
from concourse.bass_utils import run_bass_kernel_spmd

N_CORES = 8
N = 16384
D = 1024
KSLOT = 2
E = 64
SE = KSLOT * E
CAP = N // E
TOK = N // N_CORES
ITERS = 10
STAGES = [1, 2, 4]  # XOR-butterfly partner offsets
KORDER_A = [2, 0, 1, 5, 3]  # phase A: k-outer, ~DMA arrival order
KORDER_B = [4, 6, 7]  # phase B: n-outer so n-blocks finish early
N_WARM = 14  # PE p-state warm-up dummies

F32 = mybir.dt.float32

# ---- engine semaphore schedules (every op increments its engine's sem by 1
# so the OOO engine queues are race-free under the sem-watermark detector) ---

# PE program order: [phase A: no incs], n0B=1, n1B=2, [t0,t1,R0g0,t2,t3]=3..6,
# n2B=7, [t4,t5,R0g1,t6,t7]=8..11, n3B=12, [t8,t9,R0g2,t10,t11]=13..16,
# R0g3=17, t12..t15=18..21, C0=22, then R/C pairs
P_AFFN = {0: 1, 1: 2, 2: 7, 3: 16}
P_TD = {0: 3, 1: 4, 2: 5, 3: 6, 4: 8, 5: 9, 6: 10, 7: 11, 8: 12, 9: 13,
        10: 14, 11: 15, 12: 18, 13: 19, 14: 20, 15: 21}
P_R0 = 17
def P_R(r):
    return P_R0 if r == 0 else 21 + 2 * r
def P_C(r):
    return 22 + 2 * r

# DVE per round r: W=7r+1, cp=7r+2, adds=7r+3..5, recipVa/b=7r+6,7r+7
def V_W(r):
    return 7 * r + 1
def V_CP(r):
    return 7 * r + 2
def V_S(r, j):  # dve value at which stage-j send buffer of round r is ready
    return V_CP(r) + j
def V_ADD(r, j):
    return 7 * r + 3 + j
def V_V(r):
    return 7 * r + 7
V_RSUM = V_V(8) + 1  # 64


def _build_nc():
    nc = bass.Bass()

    featsT_in = nc.declare_dram_parameter("featsT", [D, TOK], F32, isOutput=False)
    centT_in = nc.declare_dram_parameter("centT", [D, SE], F32, isOutput=False)
    v0_in = nc.declare_dram_parameter("v0", [SE, 2], F32, isOutput=False)
    ident_in = nc.declare_dram_parameter("ident", [128, 128], F32, isOutput=False)

    afft_out = nc.declare_dram_parameter("afft", [SE, TOK], F32, isOutput=True)
    rsum_out = nc.declare_dram_parameter("rsum", [128, 32], F32, isOutput=True)

    from contextlib import ExitStack

    es = ExitStack()
    featsT_sb = es.enter_context(nc.sbuf_tensor("featsT_sb", [128, 8, TOK], F32))
    centT_sb = es.enter_context(nc.sbuf_tensor("centT_sb", [128, 8, SE], F32))
    e0t_sb = es.enter_context(nc.sbuf_tensor("e0t_sb", [128, TOK], F32))
    e0tm_sb = es.enter_context(nc.sbuf_tensor("e0tm_sb", [128, 16, 128], F32))
    ident_sb = es.enter_context(nc.sbuf_tensor("ident_sb", [128, 128], F32))
    v_sb = es.enter_context(nc.sbuf_tensor("v_sb", [128, 2], F32))
    w_sb = es.enter_context(nc.sbuf_tensor("w_sb", [128, 16, 2], F32))
    affT_sb = es.enter_context(nc.sbuf_tensor("affT_sb", [128, TOK], F32))
    rsum_sb = es.enter_context(nc.sbuf_tensor("rsum_sb", [128, 32], F32))
    s_sb = es.enter_context(nc.sbuf_tensor("s_sb", [128, 12], F32))  # [stage*4 + r%4]
    r_sb = es.enter_context(nc.sbuf_tensor("r_sb", [128, 6], F32))
    tot_sb = es.enter_context(nc.sbuf_tensor("tot_sb", [128, 1], F32))

    ps_aff = es.enter_context(nc.psum_tensor("ps_aff", [128, TOK], F32))
    ps_r = es.enter_context(nc.psum_tensor("ps_r", [128, 32], F32))
    ps_c = es.enter_context(nc.psum_tensor("ps_c", [128, 2], F32))
    ps_tp = es.enter_context(nc.psum_tensor("ps_tp", [128, 128], F32))
    ps_tp2 = es.enter_context(nc.psum_tensor("ps_tp2", [128, 128], F32))

    block = es.enter_context(nc.Block())
    in_sem = es.enter_context(nc.semaphore("in_sem"))
    out_sem = es.enter_context(nc.semaphore("out_sem"))
    fsems = [es.enter_context(nc.semaphore(f"fsem{k}")) for k in range(8)]
    pe_sem = es.enter_context(nc.semaphore("pe_sem"))
    act_sem = es.enter_context(nc.semaphore("act_sem"))
    dve_sem = es.enter_context(nc.semaphore("dve_sem"))
    psem = es.enter_context(nc.semaphore("psem"))
    exp_sems = [es.enter_context(nc.semaphore(f"exp{n}")) for n in range(4)]
    csems = [es.enter_context(nc.semaphore(f"cs{t}")) for t in range(16)]
    afcs = [es.enter_context(nc.semaphore(f"afc{n}")) for n in range(4)]
    lsem = es.enter_context(nc.semaphore("lsem"))
    ack_sem = es.enter_context(nc.semaphore("ack_sem"))
    rsems = [
        [es.enter_context(nc.semaphore(f"rsem{j}_{p}")) for p in range(2)]
        for j in range(3)
    ]

    with es:
        # -------- sync engine (SP): v0/ident + feats k0,k3 + output DMAs ----
        @block.sync
        def _(eng):
            eng.dma_start(out=v_sb[:], in_=v0_in[:]).then_inc(in_sem, 16)
            eng.dma_start(out=ident_sb[:], in_=ident_in[:]).then_inc(in_sem, 16)
            for k in (0, 3):
                eng.dma_start(
                    out=featsT_sb[:, k, :], in_=featsT_in[128 * k : 128 * (k + 1), :]
                ).then_inc(fsems[k], 16)
            for n in range(3):
                eng.wait_ge(afcs[n], 1)
            eng.dma_start(out=afft_out[:], in_=affT_sb[:]).then_inc(out_sem, 16)
            eng.wait_ge(dve_sem, V_RSUM)
            eng.dma_start(out=rsum_out[:], in_=rsum_sb[:]).then_inc(out_sem, 16)
            eng.wait_ge(out_sem, 32)

        # -------- scalar (ACT): centT + feats k1,k4 + exp + afft chunks -----
        @block.scalar
        def _(eng):
            src_ap = centT_in.ap().rearrange("(k p) e -> p k e", p=128)
            with nc.allow_non_contiguous_dma(reason="512B-run k-major centT load"):
                eng.dma_start(out=centT_sb[:], in_=src_ap).then_inc(in_sem, 16)
            for k in (1, 4):
                eng.dma_start(
                    out=featsT_sb[:, k, :], in_=featsT_in[128 * k : 128 * (k + 1), :]
                ).then_inc(fsems[k], 16)
            for n in range(4):  # exp chunks chasing phase-B blocks
                eng.wait_ge(pe_sem, P_AFFN[n])
                eng.activation(
                    e0t_sb[:, 512 * n : 512 * (n + 1)],
                    ps_aff[:, 512 * n : 512 * (n + 1)],
                    mybir.ActivationFunctionType.Exp,
                ).then_inc(exp_sems[n], 1)
                if n < 2:  # free banks 0/1 early for transpose scratch
                    eng.activation(
                        affT_sb[:, 512 * n : 512 * (n + 1)],
                        ps_aff[:, 512 * n : 512 * (n + 1)],
                        mybir.ActivationFunctionType.Copy,
                    ).then_inc(afcs[n], 1)
            for t in (13, 15):  # late e0tm copies (GPSIMD cannot touch PSUM)
                eng.wait_ge(pe_sem, P_TD[t])
                eng.activation(
                    e0tm_sb[:, t, :],
                    ps_aff[:, 0:128] if t == 13 else ps_tp[:],
                    mybir.ActivationFunctionType.Copy,
                ).then_inc(csems[t], 1)
            eng.activation(
                affT_sb[:, 1024:2048],
                ps_aff[:, 1024:2048],
                mybir.ActivationFunctionType.Copy,
            ).then_inc(afcs[2], 1)

        # -------- tensor engine (PE) ----------------------------------------
        @block.tensor
        def _(eng):
            # p-state warm-up on tiny tiles while feats DMAs are in flight
            eng.wait_ge(in_sem, 48)  # v0 + ident + centT
            for _ in range(N_WARM):
                eng.matmul(ps_r[0:2, 0:32], v_sb[:], ident_sb[:, 0:32],
                           start=True, stop=True)

            # phase A: k-outer over the first 5 chunks
            for i, k in enumerate(KORDER_A):
                eng.wait_ge(fsems[k], 16)
                for n in range(4):
                    eng.matmul(
                        ps_aff[:, 512 * n : 512 * (n + 1)],
                        centT_sb[:, k, :],
                        featsT_sb[:, k, 512 * n : 512 * (n + 1)],
                        start=(i == 0),
                        stop=False,
                    )
            for k in KORDER_B:
                eng.wait_ge(fsems[k], 16)

            def nblock(n):
                for j, k in enumerate(KORDER_B):
                    mm = eng.matmul(
                        ps_aff[:, 512 * n : 512 * (n + 1)],
                        centT_sb[:, k, :],
                        featsT_sb[:, k, 512 * n : 512 * (n + 1)],
                        start=False,
                        stop=(j == len(KORDER_B) - 1),
                    )
                mm.then_inc(pe_sem, 1)  # P_AFFN[n]

            def transpose(t, buf, wait=None):
                if wait is not None:
                    eng.wait_ge(*wait)
                eng.transpose(
                    buf, e0t_sb[:, 128 * t : 128 * (t + 1)], ident_sb[:]
                ).then_inc(pe_sem, 1)  # P_TD[t]

            def r_tile(t):
                return eng.matmul(
                    ps_r[:, 2 * t : 2 * (t + 1)],
                    e0t_sb[:, 128 * t : 128 * (t + 1)],
                    v_sb[:],
                    start=True,
                    stop=True,
                )

            def r_group(g):
                for t in range(4 * g, 4 * g + 4):
                    mm = r_tile(t)
                return mm

            # transpose buffers: tp/tp2 + two BANK-ALIGNED ps_aff slices freed
            # by early afft chunk copies (mid-bank matmul outputs fault on hw)
            BSEQ = ["tp", "tp2", "A", "tp", "tp2", "A", "B", "tp",
                    "tp2", "A", "B", "tp", "tp2", "A", "B", "tp"]

            def tbuf(t):
                b = BSEQ[t]
                if b == "tp":
                    return ps_tp[:]
                if b == "tp2":
                    return ps_tp2[:]
                return ps_aff[:, 0:128] if b == "A" else ps_aff[:, 512:640]

            # phase B: n-outer; transposes + R0 groups slotted into the gaps
            nblock(0)
            nblock(1)
            eng.wait_ge(exp_sems[0], 1)
            transpose(0, tbuf(0))
            transpose(1, tbuf(1))
            r_group(0)
            transpose(2, tbuf(2), (afcs[0], 1))
            transpose(3, tbuf(3), (csems[0], 1))
            nblock(2)
            eng.wait_ge(exp_sems[1], 1)
            transpose(4, tbuf(4), (csems[1], 1))
            transpose(5, tbuf(5), (csems[2], 1))
            r_group(1)
            transpose(6, tbuf(6), (afcs[1], 1))
            transpose(7, tbuf(7), (csems[3], 1))
            eng.wait_ge(exp_sems[2], 1)
            transpose(8, tbuf(8), (csems[4], 1))
            transpose(9, tbuf(9), (csems[5], 1))
            r_group(2)
            transpose(10, tbuf(10), (csems[6], 1))
            transpose(11, tbuf(11), (csems[7], 1))
            nblock(3)
            eng.wait_ge(exp_sems[3], 1)
            r_group(3).then_inc(pe_sem, 1)  # P_R0
            for t in range(12, 16):
                transpose(t, tbuf(t), (csems[t - 4], 1))

            def c_step(r):
                # slot-packed C: region [0:64] col 0 <- slot 0, [64:128] <- slot 1
                for lo, hi, s in ((0, 64, 0), (64, 128, 1)):
                    for t in range(16):
                        if r == 0 and t >= 12:
                            eng.wait_ge(csems[t], 1)
                        mm = eng.matmul(
                            ps_c[lo:hi, 0:1],
                            e0tm_sb[:, t, lo:hi],
                            w_sb[:, t, s : s + 1],
                            start=(t == 0),
                            stop=(t == 15),
                        )
                return mm

            def r_step():
                for t in range(16):
                    mm = r_tile(t)
                return mm

            for r in range(ITERS - 1):
                eng.wait_ge(dve_sem, V_W(r))
                c_step(r).then_inc(pe_sem, 1)  # P_C(r)
                eng.wait_ge(dve_sem, V_V(r))
                r_step().then_inc(pe_sem, 1)  # P_R(r+1)

        # -------- vector (DVE): e0tm copies, recips, butterfly adds ---------
        @block.vector
        def _(eng):
            BSEQ = ["tp", "tp2", "A", "tp", "tp2", "A", "B", "tp",
                    "tp2", "A", "B", "tp", "tp2", "A", "B", "tp"]

            def cbuf(t):
                b = BSEQ[t]
                if b == "tp":
                    return ps_tp[:]
                if b == "tp2":
                    return ps_tp2[:]
                return ps_aff[:, 0:128] if b == "A" else ps_aff[:, 512:640]

            for t in range(12):  # early-tile e0tm copies
                eng.wait_ge(pe_sem, P_TD[t])
                eng.tensor_copy(e0tm_sb[:, t, :], cbuf(t)).then_inc(csems[t], 1)
            # W0 = 1/R0
            eng.wait_ge(pe_sem, P_R0)
            eng.reciprocal(
                w_sb.ap().rearrange("p t s -> p (t s)"), ps_r[:, 0:32]
            ).then_inc(dve_sem, 1)  # V_W(0)
            for t in (12, 14):  # late copies
                eng.wait_ge(pe_sem, P_TD[t])
                eng.tensor_copy(e0tm_sb[:, t, :], cbuf(t)).then_inc(csems[t], 1)

            for r in range(ITERS - 1):
                par = r % 2
                par4 = r % 4
                if r >= 1:
                    eng.wait_ge(pe_sem, P_R(r))
                    eng.reciprocal(
                        w_sb.ap().rearrange("p t s -> p (t s)"), ps_r[:, 0:32]
                    ).then_inc(dve_sem, 1)  # V_W(r)
                # cpart -> s0 (slot-packed by the C-step already)
                eng.wait_ge(pe_sem, P_C(r))
                if r >= 4:
                    eng.wait_ge(ack_sem, (r - 2) // 2)  # rounds <= r-4 drained
                eng.tensor_copy(s_sb[:, par4 : par4 + 1], ps_c[:, 0:1]).then_inc(
                    dve_sem, 1
                )  # V_CP(r)
                for j in range(3):
                    eng.wait_ge(dve_sem, V_S(r, j))
                    eng.wait_ge(rsems[j][par], 2 * (r // 2 + 1))
                    dst = (
                        s_sb[:, 4 * (j + 1) + par4 : 4 * (j + 1) + par4 + 1]
                        if j < 2
                        else tot_sb[:]
                    )
                    eng.tensor_add(
                        dst,
                        s_sb[:, 4 * j + par4 : 4 * j + par4 + 1],
                        r_sb[:, 2 * j + par : 2 * j + par + 1],
                    ).then_inc(dve_sem, 1)  # V_ADD(r, j)
                # V = slot-masked 1/total
                eng.wait_ge(dve_sem, V_ADD(r, 2))
                eng.reciprocal(v_sb[0:64, 0:1], tot_sb[0:64, :]).then_inc(dve_sem, 1)
                eng.wait_ge(dve_sem, V_V(r) - 1)
                eng.reciprocal(v_sb[64:128, 1:2], tot_sb[64:128, :]).then_inc(
                    dve_sem, 1
                )  # V_V(r)

            eng.wait_ge(pe_sem, P_R(ITERS - 1))
            eng.tensor_copy(rsum_sb[:], ps_r[:, 0:32]).then_inc(dve_sem, 1)  # V_RSUM

        # -------- gpsimd (Pool): feats k2,k5,k6,k7 + late copies + butterfly
        @block.gpsimd
        def _(eng):
            from concourse import library_config

            for k in (2, 5, 6, 7):
                eng.dma_start(
                    out=featsT_sb[:, k, :], in_=featsT_in[128 * k : 128 * (k + 1), :]
                ).then_inc(fsems[k], 16)
            eng.load_library(library_config.remote_dma)
            nprep = 0
            for r in range(ITERS - 1):
                par = r % 2
                for j, d in enumerate(STAGES):
                    rdests = [None] * 8
                    rdests[d] = (0, d)
                    if nprep >= 14:  # SWDGE ring backpressure (66 descs/entry)
                        eng.wait_ge(ack_sem, -(-(nprep - 13) // 6))
                    eng.remote_dma_broadcast(
                        out_ap=r_sb[:, 2 * j + par : 2 * j + par + 1],
                        in_ap=s_sb[:, 4 * j + r % 4 : 4 * j + r % 4 + 1],
                        remote_sem=rsems[j][par],
                        local_sem=lsem,
                        rdests=rdests,
                    ).then_inc(psem, 1)
                    nprep += 1
                    eng.wait_ge(psem, nprep)
                    eng.wait_ge(dve_sem, V_S(r, j))
                    eng.trigger_dma(1)
                    if j == 2 and r % 2 == 1:
                        # both rounds of this parity pair drained -> ack
                        eng.wait_ge(lsem, 16 * nprep).then_inc(ack_sem, 1)

    return nc


_CACHE = {}


def _get_nc():
    if "nc" not in _CACHE:
        nc = _build_nc()
        # Raw Bass skips Bacc's codegen_inst_isa pass; without it the NEFF
        # compiler sees empty .instr on the extended-ISA (remote DMA, library
        # load) instructions and fails with "ISA wrong length".
        from concourse.library_overlay import lower_extended_insts

        lower_extended_insts(nc)
        _CACHE["nc"] = nc
    return _CACHE["nc"]


def make_in_maps(input_features, expert_centroids):
    feats = np.ascontiguousarray(
        np.asarray(input_features, dtype=np.float32).reshape(-1, D)
    )
    cent = np.asarray(expert_centroids, dtype=np.float32).reshape(SE, D)

    featsT = np.ascontiguousarray(feats.T)
    centT = np.ascontiguousarray(cent.T)
    ident = np.eye(128, dtype=np.float32)
    v0 = np.zeros((SE, 2), np.float32)
    v0[0:64, 0] = 1.0
    v0[64:128, 1] = 1.0

    in_maps = []
    for c in range(N_CORES):
        in_maps.append(
            {
                "featsT": np.ascontiguousarray(featsT[:, TOK * c : TOK * (c + 1)]),
                "centT": centT,
                "ident": ident,
                "v0": v0,
            }
        )
    return in_maps


def kernel(input_features: np.ndarray, expert_centroids: np.ndarray):
    in_maps = make_in_maps(input_features, expert_centroids)
    nc = _get_nc()
    res = run_bass_kernel_spmd(nc, in_maps, list(range(N_CORES)))

    afft = np.concatenate([res.results[c]["afft"] for c in range(N_CORES)], axis=1)
    # rsum[c][p, 2t+s] = R_s[token c*2048 + t*128 + p]
    lnr = np.empty((KSLOT, N), np.float64)
    for c in range(N_CORES):
        rs = np.asarray(res.results[c]["rsum"], np.float64).reshape(128, 16, 2)
        for s in range(KSLOT):
            lnr[s, c * TOK : (c + 1) * TOK] = np.log(rs[:, :, s]).T.reshape(-1)

    A = afft.reshape(KSLOT, E, N)
    Z = A.astype(np.float64) - lnr[:, None, :]
    idx = np.empty((KSLOT, E, CAP), np.int32)
    vals = np.empty((KSLOT, E, CAP), np.float32)
    for k in range(KSLOT):
        for e in range(E):
            col = Z[k, e]
            part = np.sort(np.argpartition(-col, CAP - 1)[:CAP])
            order = part[np.argsort(-col[part], kind="stable")]
            idx[k, e] = order.astype(np.int32)
            vals[k, e] = A[k, e, order]
    return idx, vals
